# revision 2
# baseline (speedup 1.0000x reference)
"""Bass/Trainium2 kernel for 7x7 valid cross-correlation on a 8192x8192 fp32 image.

Sharding: output rows split across 8 NeuronCores (spatial data-parallel).
Each core receives an overlapping input slice (6 halo rows), so no
device-to-device communication is needed.

Per-core compute: conv2d is mapped onto the TensorEngine as 7 PSUM-accumulated
matmuls per output tile. For column tap j, the stationary operand is a banded
Toeplitz matrix B_j[k, m] = weight[k-m, j] (0 <= k-m < 7), built on the host
from the 7x7 weight. Contraction runs over 128 input rows; the moving operand
is the image tile with its free-dim (columns) shifted by j. One matmul yields
122 valid output rows x 512 output columns; summing the 7 taps in PSUM gives
the full 2D convolution.
"""

import numpy as np

import concourse.bacc as bacc
import concourse.tile as tile
import concourse.mybir as mybir
from concourse.bass_utils import run_bass_kernel_spmd

H = W = 8192
KH = KW = 7
OH = OW = H - KH + 1  # 8186

N_CORES = 8
ROWS_PER_CORE = 1024          # output rows per core (last 6 of core 7 are pad)
IN_ROWS = ROWS_PER_CORE + KH - 1  # 1030 input rows per core

GROUP = 122                   # valid output rows per matmul group (128 - KH + 1)
NTILE = 512                   # output columns per PSUM bank
# Row-group starts within a core's 1024 output rows (last group overlaps).
GROUP_STARTS = [122 * g for g in range(8)] + [ROWS_PER_CORE - GROUP]  # ...,854,902
# Column-tile starts (last tile overlaps; its first 6 columns are not stored).
COL_STARTS = [512 * t for t in range(15)] + [OW - NTILE]  # ..., 7168, 7674

MM_DT = mybir.dt.float32r    # full-rate PE for N>=256


def _build_nc():
    nc = bacc.Bacc(
        "TRN2", target_bir_lowering=False, debug=False, num_devices=N_CORES
    )
    x = nc.dram_tensor("x", [IN_ROWS, W], MM_DT, kind="ExternalInput").ap()
    B = nc.dram_tensor("B", [128, KW * 128], MM_DT, kind="ExternalInput").ap()
    bias = nc.dram_tensor("bias", [128, 1], mybir.dt.float32, kind="ExternalInput").ap()
    y = nc.dram_tensor(
        "y", [ROWS_PER_CORE, OW], mybir.dt.float32, kind="ExternalOutput"
    ).ap()

    with tile.TileContext(nc) as tc:
        with (
            tc.tile_pool(name="consts", bufs=1) as consts,
            tc.tile_pool(name="xin", bufs=2) as xin,
            tc.tile_pool(name="outs", bufs=4) as outs,
            tc.tile_pool(name="psum", bufs=4, space="PSUM") as psum_pool,
        ):
            B_sb = consts.tile([128, KW * 128], MM_DT)
            nc.sync.dma_start(B_sb[:], B[:])
            bias_sb = consts.tile([128, 1], mybir.dt.float32)
            nc.sync.dma_start(bias_sb[:], bias[:])

            for g0 in GROUP_STARTS:
                x_sb = xin.tile([128, W], MM_DT)
                nc.sync.dma_start(x_sb[:], x[g0 : g0 + 128, :])
                for c0 in COL_STARTS:
                    ps = psum_pool.tile([128, NTILE], mybir.dt.float32)
                    for j in range(KW):
                        nc.tensor.matmul(
                            ps[:, :],
                            B_sb[:, j * 128 : (j + 1) * 128],
                            x_sb[:, c0 + j : c0 + j + NTILE],
                            start=(j == 0),
                            stop=(j == KW - 1),
                        )
                    o_sb = outs.tile([128, NTILE], mybir.dt.float32)
                    nc.vector.tensor_scalar_add(
                        o_sb[0:GROUP, :], ps[0:GROUP, :], bias_sb[0:GROUP, 0:1]
                    )
                    if c0 == COL_STARTS[-1]:
                        # overlapped tail tile: skip the 6 recomputed columns
                        nc.scalar.dma_start(
                            y[g0 : g0 + GROUP, c0 + 6 : c0 + NTILE],
                            o_sb[0:GROUP, 6:NTILE],
                        )
                    else:
                        nc.scalar.dma_start(
                            y[g0 : g0 + GROUP, c0 : c0 + NTILE], o_sb[0:GROUP, :]
                        )

    nc.compile()
    return nc


_NC_CACHE = None


def _get_nc():
    global _NC_CACHE
    if _NC_CACHE is None:
        _NC_CACHE = _build_nc()
    return _NC_CACHE


def kernel(x: np.ndarray, weight: np.ndarray, bias: np.ndarray) -> np.ndarray:
    x = np.ascontiguousarray(x, dtype=np.float32)
    weight = np.asarray(weight, dtype=np.float32)
    bias = np.asarray(bias, dtype=np.float32)

    # Banded Toeplitz blocks: B[k, j*128 + m] = weight[k-m, j], 0 <= k-m < KH.
    B = np.zeros((128, KW * 128), dtype=np.float32)
    m = np.arange(GROUP)
    for j in range(KW):
        for d in range(KH):
            B[m + d, j * 128 + m] = weight[d, j]

    bias_bcast = np.full((128, 1), bias[0], dtype=np.float32)

    # Pad 6 zero rows so every core's input slice has identical shape.
    x_pad = np.concatenate([x, np.zeros((KH - 1, W), dtype=np.float32)], axis=0)
    in_maps = [
        {
            "x": np.ascontiguousarray(
                x_pad[c * ROWS_PER_CORE : c * ROWS_PER_CORE + IN_ROWS]
            ),
            "B": B,
            "bias": bias_bcast,
        }
        for c in range(N_CORES)
    ]

    nc = _get_nc()
    res = run_bass_kernel_spmd(nc, in_maps, core_ids=list(range(N_CORES)))
    full = np.concatenate([res.results[c]["y"] for c in range(N_CORES)], axis=0)
    return full[:OH]


# revision 3
# speedup vs baseline: 1.4901x; 1.4901x over previous
"""Bass/Trainium2 kernel for 7x7 valid cross-correlation on a 8192x8192 fp32 image.

Sharding: output rows split across 8 NeuronCores (spatial data-parallel).
Each core receives an overlapping input slice (6 halo rows), so no
device-to-device communication is needed.

Per-core compute: conv2d is mapped onto the TensorEngine as 7 PSUM-accumulated
matmuls per output tile. For column tap j, the stationary operand is a banded
Toeplitz matrix B_j[k, m] = weight[k-m, j] (0 <= k-m < 7), built on the host
from the 7x7 weight. Contraction runs over 128 input rows; the moving operand
is the image tile with its free-dim (columns) shifted by j. One matmul yields
122 valid output rows x 512 output columns; summing the 7 taps in PSUM gives
the full 2D convolution. float32r keeps the PE at one column per cycle while
staying within ~2e-4 of the fp32 reference.

Per core: 8 full row-groups of 122 output rows plus one trimmed 48-row group
(= 1024 rows), 16 column tiles of 512. Input loads are split into 16 chunks so
column tiles start as soon as their columns land; loads run on the sync-engine
HWDGE ring and stores on the scalar-engine ring.
"""

import numpy as np

import concourse.bacc as bacc
import concourse.tile as tile
import concourse.mybir as mybir
from concourse.bass_utils import run_bass_kernel_spmd

H = W = 8192
KH = KW = 7
OH = OW = H - KH + 1  # 8186

N_CORES = 8
ROWS_PER_CORE = 1024          # output rows per core (last 6 of core 7 are pad)
IN_ROWS = ROWS_PER_CORE + KH - 1  # 1030 input rows per core

GROUP = 122                   # valid output rows per full matmul group
NTILE = 512                   # output columns per PSUM bank
SPLIT_LOAD = 16               # input DMA chunks per row-group
# 8 full groups + one trimmed 48-row group covering rows 976..1023.
GROUP_STARTS = [122 * g for g in range(8)] + [976]
# Column-tile starts (last tile overlaps; its first 6 columns are not stored).
COL_STARTS = [512 * t for t in range(15)] + [OW - NTILE]  # ..., 7168, 7674

MM_DT = mybir.dt.float32r    # full-rate PE for N>=256


def _build_nc():
    nc = bacc.Bacc(
        "TRN2", target_bir_lowering=False, debug=False, num_devices=N_CORES
    )
    x = nc.dram_tensor("x", [IN_ROWS, W], MM_DT, kind="ExternalInput").ap()
    B = nc.dram_tensor("B", [128, KW * 128], MM_DT, kind="ExternalInput").ap()
    bias = nc.dram_tensor("bias", [128, 1], mybir.dt.float32, kind="ExternalInput").ap()
    y = nc.dram_tensor(
        "y", [ROWS_PER_CORE, OW], mybir.dt.float32, kind="ExternalOutput"
    ).ap()

    with tile.TileContext(nc) as tc:
        with (
            tc.tile_pool(name="consts", bufs=1) as consts,
            tc.tile_pool(name="xin", bufs=3) as xin,
            tc.tile_pool(name="outs", bufs=8) as outs,
            tc.tile_pool(name="psum", bufs=8, space="PSUM") as psum_pool,
        ):
            B_sb = consts.tile([128, KW * 128], MM_DT)
            nc.sync.dma_start(B_sb[:], B[:])
            bias_sb = consts.tile([128, 1], mybir.dt.float32)
            nc.sync.dma_start(bias_sb[:], bias[:])

            for g0 in GROUP_STARTS:
                grows = GROUP if g0 != GROUP_STARTS[-1] else ROWS_PER_CORE - 976
                krows = grows + KH - 1
                mcols = 128 if grows == GROUP else grows

                x_sb = xin.tile([128, W], MM_DT)
                step = W // SPLIT_LOAD
                for s in range(SPLIT_LOAD):
                    nc.sync.dma_start(
                        x_sb[0:krows, s * step : (s + 1) * step],
                        x[g0 : g0 + krows, s * step : (s + 1) * step],
                    )
                for c0 in COL_STARTS:
                    ps = psum_pool.tile([128, NTILE], mybir.dt.float32)
                    for j in range(KW):
                        nc.tensor.matmul(
                            ps[0:mcols, :],
                            B_sb[0:krows, j * 128 : j * 128 + mcols],
                            x_sb[0:krows, c0 + j : c0 + j + NTILE],
                            start=(j == 0),
                            stop=(j == KW - 1),
                        )
                    o_sb = outs.tile([128, NTILE], mybir.dt.float32)
                    nc.vector.tensor_scalar_add(
                        o_sb[0:grows, :], ps[0:grows, :], bias_sb[0:grows, 0:1]
                    )
                    if c0 == COL_STARTS[-1]:
                        # overlapped tail tile: skip the 6 recomputed columns
                        nc.scalar.dma_start(
                            y[g0 : g0 + grows, c0 + 6 : c0 + NTILE],
                            o_sb[0:grows, 6:NTILE],
                        )
                    else:
                        nc.scalar.dma_start(
                            y[g0 : g0 + grows, c0 : c0 + NTILE], o_sb[0:grows, :]
                        )

    nc.compile()
    return nc


_NC_CACHE = None


def _get_nc():
    global _NC_CACHE
    if _NC_CACHE is None:
        _NC_CACHE = _build_nc()
    return _NC_CACHE


def make_in_maps(x, weight, bias):
    x = np.ascontiguousarray(x, dtype=np.float32)
    weight = np.asarray(weight, dtype=np.float32)
    bias = np.asarray(bias, dtype=np.float32)

    # Banded Toeplitz blocks: B[k, j*128 + m] = weight[k-m, j], 0 <= k-m < KH.
    B = np.zeros((128, KW * 128), dtype=np.float32)
    m = np.arange(GROUP)
    for j in range(KW):
        for d in range(KH):
            B[m + d, j * 128 + m] = weight[d, j]

    bias_bcast = np.full((128, 1), bias[0], dtype=np.float32)

    # Pad 6 zero rows so every core's input slice has identical shape.
    x_pad = np.concatenate([x, np.zeros((KH - 1, W), dtype=np.float32)], axis=0)
    return [
        {
            "x": np.ascontiguousarray(
                x_pad[c * ROWS_PER_CORE : c * ROWS_PER_CORE + IN_ROWS]
            ),
            "B": B,
            "bias": bias_bcast,
        }
        for c in range(N_CORES)
    ]


def kernel(x: np.ndarray, weight: np.ndarray, bias: np.ndarray) -> np.ndarray:
    in_maps = make_in_maps(x, weight, bias)
    nc = _get_nc()
    res = run_bass_kernel_spmd(nc, in_maps, core_ids=list(range(N_CORES)))
    full = np.concatenate([res.results[c]["y"] for c in range(N_CORES)], axis=0)
    return full[:OH]


# revision 7
# speedup vs baseline: 1.5496x; 1.0399x over previous
"""Bass/Trainium2 kernel for 7x7 valid cross-correlation on a 8192x8192 fp32 image.

Sharding: output rows split across 8 NeuronCores (spatial data-parallel).
Each core receives an overlapping input slice (6 halo rows), so no
device-to-device communication is needed.

Per-core compute: conv2d is mapped onto the TensorEngine as 7 PSUM-accumulated
matmuls per output tile. For column tap j, the stationary operand is a banded
Toeplitz matrix B_j[k, m] = weight[k-m, j] (0 <= k-m < 7), built on the host
from the 7x7 weight. Contraction runs over 128 input rows; the moving operand
is the image tile with its free-dim (columns) shifted by j. One matmul yields
122 valid output rows x 512 output columns; summing the 7 taps in PSUM gives
the full 2D convolution. float32r keeps the PE at one column per cycle while
staying within ~2e-4 of the fp32 reference.

Per core: 8 full row-groups of 122 output rows plus one trimmed 48-row group
(= 1024 rows), 16 column tiles of 512. Input loads are split into 16 chunks so
column tiles start as soon as their columns land; loads run on the sync-engine
HWDGE ring and stores on the scalar-engine ring.
"""

import numpy as np

import concourse.bacc as bacc
import concourse.tile as tile
import concourse.mybir as mybir
from concourse.bass_utils import run_bass_kernel_spmd

H = W = 8192
KH = KW = 7
OH = OW = H - KH + 1  # 8186

N_CORES = 8
ROWS_PER_CORE = 1024          # output rows per core (last 6 of core 7 are pad)
IN_ROWS = ROWS_PER_CORE + KH - 1  # 1030 input rows per core

GROUP = 122                   # valid output rows per full matmul group
NTILE = 512                   # output columns per PSUM bank
SPLIT_LOAD = 16               # input DMA chunks per row-group
# 8 full groups + one trimmed 48-row group covering rows 976..1023.
GROUP_STARTS = [122 * g for g in range(8)] + [976]
# Column-tile starts (last tile overlaps; its first 6 columns are not stored).
COL_STARTS = [512 * t for t in range(15)] + [OW - NTILE]  # ..., 7168, 7674

MM_DT = mybir.dt.float32r    # full-rate PE for N>=256


def _build_nc():
    nc = bacc.Bacc(
        "TRN2", target_bir_lowering=False, debug=False, num_devices=N_CORES
    )
    x = nc.dram_tensor("x", [IN_ROWS, W], MM_DT, kind="ExternalInput").ap()
    B = nc.dram_tensor("B", [128, KW * 128], MM_DT, kind="ExternalInput").ap()
    bias = nc.dram_tensor("bias", [128, 1], mybir.dt.float32, kind="ExternalInput").ap()
    y = nc.dram_tensor(
        "y", [ROWS_PER_CORE, OW], mybir.dt.float32, kind="ExternalOutput"
    ).ap()

    with tile.TileContext(nc) as tc:
        with (
            tc.tile_pool(name="consts", bufs=1) as consts,
            tc.tile_pool(name="xin", bufs=2) as xin,
            tc.tile_pool(name="outs", bufs=8) as outs,
            tc.tile_pool(name="psum", bufs=8, space="PSUM") as psum_pool,
        ):
            # Warm the PE (HAM clock gate) with dummy matmuls on a zeroed
            # tile while the first input tiles stream in. fp32r memset is
            # invalid ISA, so memset fp32 then cast-copy (= fp32r rounding).
            wu32 = consts.tile([128, NTILE], mybir.dt.float32)
            nc.vector.memset(wu32[:], 0.0)
            wu = consts.tile([128, NTILE], MM_DT)
            nc.vector.tensor_copy(wu[:], wu32[:])
            wps = psum_pool.tile(
                [128, NTILE], mybir.dt.float32, name="wps", tag="ps"
            )
            for _ in range(8):
                nc.tensor.matmul(
                    wps[:, :], wu[:, 0:128], wu[:, :], start=True, stop=True
                )

            # B/bias ride the scalar HWDGE ring; x loads keep the sync ring.
            B_sb = consts.tile([128, KW * 128], MM_DT)
            nc.scalar.dma_start(B_sb[:], B[:])
            bias_sb = consts.tile([128, 1], mybir.dt.float32)
            nc.scalar.dma_start(bias_sb[:], bias[:])

            for g0 in GROUP_STARTS:
                grows = GROUP if g0 != GROUP_STARTS[-1] else ROWS_PER_CORE - 976
                krows = grows + KH - 1
                mcols = 128 if grows == GROUP else grows

                x_sb = xin.tile([128, W], MM_DT)
                step = W // SPLIT_LOAD
                for s in range(SPLIT_LOAD):
                    nc.sync.dma_start(
                        x_sb[0:krows, s * step : (s + 1) * step],
                        x[g0 : g0 + krows, s * step : (s + 1) * step],
                    )
                for c0 in COL_STARTS:
                    ps = psum_pool.tile(
                        [128, NTILE], mybir.dt.float32, name="ps", tag="ps"
                    )
                    for j in range(KW):
                        nc.tensor.matmul(
                            ps[0:mcols, :],
                            B_sb[0:krows, j * 128 : j * 128 + mcols],
                            x_sb[0:krows, c0 + j : c0 + j + NTILE],
                            start=(j == 0),
                            stop=(j == KW - 1),
                        )
                    o_sb = outs.tile([128, NTILE], mybir.dt.float32)
                    nc.vector.tensor_scalar_add(
                        o_sb[0:grows, :], ps[0:grows, :], bias_sb[0:grows, 0:1]
                    )
                    if c0 == COL_STARTS[-1]:
                        # overlapped tail tile: skip the 6 recomputed columns
                        nc.scalar.dma_start(
                            y[g0 : g0 + grows, c0 + 6 : c0 + NTILE],
                            o_sb[0:grows, 6:NTILE],
                        )
                    else:
                        nc.scalar.dma_start(
                            y[g0 : g0 + grows, c0 : c0 + NTILE], o_sb[0:grows, :]
                        )

    nc.compile()
    return nc


_NC_CACHE = None


def _get_nc():
    global _NC_CACHE
    if _NC_CACHE is None:
        _NC_CACHE = _build_nc()
    return _NC_CACHE


def make_in_maps(x, weight, bias):
    x = np.ascontiguousarray(x, dtype=np.float32)
    weight = np.asarray(weight, dtype=np.float32)
    bias = np.asarray(bias, dtype=np.float32)

    # Banded Toeplitz blocks: B[k, j*128 + m] = weight[k-m, j], 0 <= k-m < KH.
    B = np.zeros((128, KW * 128), dtype=np.float32)
    m = np.arange(GROUP)
    for j in range(KW):
        for d in range(KH):
            B[m + d, j * 128 + m] = weight[d, j]

    bias_bcast = np.full((128, 1), bias[0], dtype=np.float32)

    # Pad 6 zero rows so every core's input slice has identical shape.
    x_pad = np.concatenate([x, np.zeros((KH - 1, W), dtype=np.float32)], axis=0)
    return [
        {
            "x": np.ascontiguousarray(
                x_pad[c * ROWS_PER_CORE : c * ROWS_PER_CORE + IN_ROWS]
            ),
            "B": B,
            "bias": bias_bcast,
        }
        for c in range(N_CORES)
    ]


def kernel(x: np.ndarray, weight: np.ndarray, bias: np.ndarray) -> np.ndarray:
    in_maps = make_in_maps(x, weight, bias)
    nc = _get_nc()
    res = run_bass_kernel_spmd(nc, in_maps, core_ids=list(range(N_CORES)))
    full = np.concatenate([res.results[c]["y"] for c in range(N_CORES)], axis=0)
    return full[:OH]
